# revision 18
# baseline (speedup 1.0000x reference)
"""Trainium2 Bass kernel for nn_CrossAttention (B=8, L=1024, QD=1024, KVD=768, H=16).

Sharding: data-parallel over batch across the 8 NeuronCores (1 batch row each).
Per-core pipeline (all bf16 matmuls, fp32 accumulation / layernorm):
  A) prologue with NO DRAM bounce and NO DMA transposes. Two parallel HBM
     streams: weights fp32 on the sync HWDGE ring (PE fp32 transposes, cast
     to bf16 in the DVE psum eviction); activations q/k/v DMA-cast fp32->bf16
     on SWDGE, then PE bf16 transposes. Transposed layouts are row-tile-major
     [P, rowtile, ct, 128]. q's natural bf16 copy persists and serves as the
     residual (bf16 residual, well within tolerance).
  B) projections: qhT/khT (per-partition bias via tensor_scalar), vh natural
     with bias added on DVE during psum eviction (bv broadcast tile). B1 is
     lh-outer so it can start on half of qT.
  C) attention per head pair: scoresT = khT.T @ qhT, exp with mask+scale folded
     into the ACT pass, attnV with [ones|vh] stationary via a strided AP into
     a shared-ones vh tile (psum rows 0:64 = replicated denominator, 64:128 =
     o); fast approx reciprocal + multiply on DVE. attnV lags one pair behind
     scores+exp. Wo's fp32 PE transposes share the b3 PSUM slot mid-attention.
  D) out-projection from oT stationary + rank-1 bias, bf16 residual + layernorm
     with per-512-half eviction/bn_stats to shorten the tail.
"""

import numpy as np

import concourse.bass as bass
import concourse.mybir as mybir
import concourse.tile as tile
from concourse import bacc
from concourse.bass_utils import run_bass_kernel_spmd
from concourse.masks import make_identity

F32 = mybir.dt.float32
BF16 = mybir.dt.bfloat16
U8 = mybir.dt.uint8

B = 8
L = 1024
C = 1024      # QD
KV = 768      # KVD
H = 16
DH = 64
P = 128
LT = L // P          # 8 l-tiles
CT = C // P          # 8 contraction tiles (model dim)
KT = KV // P         # 6 contraction tiles (kv dim)
DT = C // P          # 8 d-tiles
NH = C // 512        # 2 free-dim halves (N=512 per PSUM bank)
VW = (H // 2) * 3 * DH   # 1536: 8 pair-groups of [vh_even | ones | vh_odd]
SCALE = DH ** -0.5
EPS = 1e-5
MASK_NEG = -30000.0

Exp = mybir.ActivationFunctionType.Exp
Sqrt = mybir.ActivationFunctionType.Sqrt
Identity = mybir.ActivationFunctionType.Identity
MULT = mybir.AluOpType.mult
ADD = mybir.AluOpType.add

TRACE = False
LAST_RESULT = None
_CACHE = {}


def _bcast_ap(handle, parts):
    apx = handle[:]
    return bass.AP(tensor=apx.tensor, offset=apx.offset,
                   ap=[[0, parts]] + [list(x) for x in apx.ap])


def _rowtiles(hnd, r0, nt, cols):
    # DRAM AP [p, j, c] = hnd[r0 + j*P + p, c] for j in [0, nt)
    return hnd[r0:r0 + nt * P, :].rearrange("(j p) c -> p j c", p=P)


def build(apply_gb=False):
    nc = bacc.Bacc("TRN2", target_bir_lowering=False)

    q_in = nc.dram_tensor("q", [L, C], F32, kind="ExternalInput")
    k_in = nc.dram_tensor("k", [L, KV], F32, kind="ExternalInput")
    v_in = nc.dram_tensor("v", [L, KV], F32, kind="ExternalInput")
    m_in = nc.dram_tensor("key_padding_mask", [L], U8, kind="ExternalInput")
    wq_in = nc.dram_tensor("Wq", [C, C], F32, kind="ExternalInput")
    bq_in = nc.dram_tensor("bq", [C], F32, kind="ExternalInput")
    wk_in = nc.dram_tensor("Wk", [C, KV], F32, kind="ExternalInput")
    bk_in = nc.dram_tensor("bk", [C], F32, kind="ExternalInput")
    wv_in = nc.dram_tensor("Wv", [C, KV], F32, kind="ExternalInput")
    bv_in = nc.dram_tensor("bv", [C], F32, kind="ExternalInput")
    wo_in = nc.dram_tensor("Wo", [C, C], F32, kind="ExternalInput")
    bo_in = nc.dram_tensor("bo", [C], F32, kind="ExternalInput")
    gamma_in = nc.dram_tensor("gamma", [C], F32, kind="ExternalInput")
    beta_in = nc.dram_tensor("beta", [C], F32, kind="ExternalInput")
    y_out = nc.dram_tensor("y", [L, C], F32, kind="ExternalOutput")

    with tile.TileContext(nc) as tc:
        with (
            tc.tile_pool(name="flat", bufs=1) as flat,
            tc.tile_pool(name="cst", bufs=1) as cst,
            tc.tile_pool(name="poolV", bufs=1) as poolV,
        ):
            qhT = flat.tile([P, DT, L], BF16)           # d on partitions
            khT = flat.tile([P, DT, L], BF16)
            q_nat = flat.tile([P, LT, C], BF16)         # natural q, also residual
            vT = poolV.tile([P, LT, KT, P], BF16)
            WvT = poolV.tile([P, DT, KT, P], BF16)

            # ---------------- prologue scope A
            with (
                tc.tile_pool(name="poolQ", bufs=1) as poolQ,
                tc.tile_pool(name="poolK", bufs=1) as poolK,
                tc.tile_pool(name="wstg", bufs=3) as wstg,
                tc.tile_pool(name="kvstg", bufs=2) as kvstg,
                tc.tile_pool(name="psum_b", bufs=2, space="PSUM") as psum_b,
                tc.tile_pool(name="pst32", bufs=2, space="PSUM") as pst32,
                tc.tile_pool(name="pstb", bufs=2, space="PSUM") as pstb,
            ):
                qT = poolQ.tile([P, LT, CT, P], BF16)
                WqT = poolQ.tile([P, DT, CT, P], BF16)
                kT = poolK.tile([P, LT, KT, P], BF16)
                WkT = poolK.tile([P, DT, KT, P], BF16)

                # tiny consts first
                bq_sb = cst.tile([P, DT], F32)
                nc.gpsimd.dma_start(bq_sb, bq_in[:].rearrange("(t p) -> p t", p=P))
                bk_sb = cst.tile([P, DT], F32)
                nc.gpsimd.dma_start(bk_sb, bk_in[:].rearrange("(t p) -> p t", p=P))
                mask_u8 = cst.tile([P, LT], U8)
                nc.gpsimd.dma_start(mask_u8, m_in[:].rearrange("(t p) -> p t", p=P))
                mask_bias = cst.tile([P, LT], F32)
                nc.vector.tensor_copy(mask_bias, mask_u8)
                nc.vector.tensor_scalar(mask_bias, mask_bias, -MASK_NEG, MASK_NEG,
                                        MULT, ADD)
                ident = cst.tile([P, P], BF16)
                make_identity(nc, ident)
                ident32 = cst.tile([P, P], F32)
                nc.vector.tensor_copy(ident32, ident)
                ones_row = cst.tile([1, P], BF16)
                nc.vector.memset(ones_row, 1.0)
                eps_sb = cst.tile([P, 1], F32)
                nc.vector.memset(eps_sb, EPS)
                bvb = cst.tile([P, C], F32)
                nc.gpsimd.dma_start(bvb, _bcast_ap(bv_in, P))
                bo_bf = cst.tile([1, C], BF16)
                nc.gpsimd.dma_start(bo_bf, bo_in[:].rearrange("(a c) -> a c", a=1))
                if apply_gb:
                    gamma_b = cst.tile([P, C], F32)
                    nc.gpsimd.dma_start(gamma_b, _bcast_ap(gamma_in, P))
                    beta_b = cst.tile([P, C], F32)
                    nc.gpsimd.dma_start(beta_b, _bcast_ap(beta_in, P))
                else:
                    gamma_b = beta_b = None

                # ---- SWDGE activation casts fp32->bf16 (q first, then k, v)
                for ch in range(2):
                    nc.gpsimd.dma_start(q_nat[:, ch * 4:(ch + 1) * 4, :],
                                        _rowtiles(q_in, ch * 4 * P, 4, C))
                k_nat = []
                for ch in range(2):
                    st = kvstg.tile([P, 4, KV], BF16, name=f"st_k{ch}", tag="kv")
                    nc.gpsimd.dma_start(st, _rowtiles(k_in, ch * 4 * P, 4, KV))
                    k_nat.append(st)
                v_nat = []
                for ch in range(2):
                    st = kvstg.tile([P, 4, KV], BF16, name=f"st_v{ch}", tag="kv")
                    nc.gpsimd.dma_start(st, _rowtiles(v_in, ch * 4 * P, 4, KV))
                    v_nat.append(st)

                # ---- HWDGE (sync) weight loads, fp32 natural
                def load_w32(nm, hnd, rows, cols):
                    tiles = []
                    for ch in range(rows // P // 2):
                        st = wstg.tile([P, 2, cols], F32, name=f"w_{nm}{ch}",
                                       tag="wstg")
                        nc.sync.dma_start(st, _rowtiles(hnd, ch * 2 * P, 2, cols))
                        tiles.append(st)
                    return tiles

                wq_nat = load_w32("wq", wq_in, C, C)
                wk_nat = load_w32("wk", wk_in, C, KV)
                wv_nat = load_w32("wv", wv_in, C, KV)

                def transp32(dstT, nat_tiles, ctn, jts):
                    # fp32 PE transposes (2 cyc/row), cast to bf16 in eviction
                    for jt in jts:
                        src = nat_tiles[jt // 2][:, jt % 2, :]
                        ps = pst32.tile([P, ctn, P], F32, tag="p32")
                        for ct in range(ctn):
                            nc.tensor.matmul(ps[:, ct, :],
                                             src[:, ct * P:(ct + 1) * P],
                                             ident32, is_transpose=True)
                        nc.vector.tensor_copy(dstT[:, jt, :, :], ps)

                def transpbf(dstT, src_of_jt, ctn, jts):
                    for jt in jts:
                        src = src_of_jt(jt)
                        ps = pstb.tile([P, ctn, P], BF16, tag="pbf")
                        for ct in range(ctn):
                            nc.tensor.transpose(ps[:, ct, :],
                                                src[:, ct * P:(ct + 1) * P], ident)
                        nc.vector.tensor_copy(dstT[:, jt, :, :], ps)

                def b_proj(dst, wT, xT, ctn, bias, lh):
                    for dt in range(DT):
                        ps = psum_b.tile([P, 512], F32, tag="ps")
                        for ct in range(ctn):
                            nc.tensor.matmul(ps, wT[:, dt, ct, :],
                                             xT[:, lh * 4:(lh + 1) * 4, ct, :],
                                             start=(ct == 0), stop=(ct == ctn - 1))
                        nc.vector.tensor_scalar_add(
                            dst[:, dt, lh * 512:(lh + 1) * 512], ps,
                            bias[:, dt:dt + 1])

                # ---- PE stream
                transp32(WqT, wq_nat, CT, range(8))
                transpbf(qT, lambda jt: q_nat[:, jt, :], CT, range(4))
                b_proj(qhT, WqT, qT, CT, bq_sb, 0)      # B1 lh=0
                transpbf(qT, lambda jt: q_nat[:, jt, :], CT, range(4, 8))
                b_proj(qhT, WqT, qT, CT, bq_sb, 1)      # B1 lh=1
                transp32(WkT, wk_nat, KT, range(8))
                transpbf(kT, lambda jt: k_nat[jt // 4][:, jt % 4, :], KT, range(4))
                b_proj(khT, WkT, kT, KT, bk_sb, 0)      # B2 lh=0
                transpbf(kT, lambda jt: k_nat[jt // 4][:, jt % 4, :], KT, range(4, 8))
                b_proj(khT, WkT, kT, KT, bk_sb, 1)      # B2 lh=1
                transp32(WvT, wv_nat, KT, range(8))
                transpbf(vT, lambda jt: v_nat[jt // 4][:, jt % 4, :], KT, range(8))

            # ---------------- scope B0: reuses prologue space, lives to the end
            with tc.tile_pool(name="b0", bufs=1) as b0:
                WoT = b0.tile([P, DT, CT, P], BF16)
                oT = b0.tile([P, DT, L], BF16)
                vh = b0.tile([P, LT, VW], BF16)   # [ones(64) | 16 x vh(64)]
                wo_nat = [b0.tile([P, 1, C], F32, name=f"w_wo{ch}", tag="wo",
                                  bufs=4)
                          for ch in range(8)]
                for ch in range(8):
                    nc.sync.dma_start(wo_nat[ch], _rowtiles(wo_in, ch * P, 1, C))

                vh_base = vh[:]

                def vh_stat(mt, h):
                    # contiguous [128 keys, 128]: even head -> [vh|ones],
                    # odd head -> [ones|vh] within its pair's 192-col group
                    c0 = 192 * (h // 2) + (0 if h % 2 == 0 else DH)
                    return vh[:, mt, c0:c0 + 2 * DH]

                ones_ap = bass.AP(tensor=vh_base.tensor,
                                  offset=vh_base.offset + DH,
                                  ap=[list(vh_base.ap[0]), [VW, LT],
                                      [3 * DH, H // 2], [1, DH]])
                nc.vector.memset(ones_ap, 1.0)

                with (
                    tc.tile_pool(name="ptp", bufs=24) as ptp,
                    tc.tile_pool(name="recp", bufs=4) as recp,
                    tc.tile_pool(name="psum_sc", bufs=2, space="PSUM") as psum_sc,
                    tc.tile_pool(name="psum_av", bufs=3, space="PSUM") as psum_av,
                    tc.tile_pool(name="psum_b3", bufs=1, space="PSUM") as psum_b3,
                ):
                    pts = {}

                    def scores_exp(pair):
                        for mt in range(LT):
                            sc = []
                            for hh in range(2):
                                s = psum_sc.tile([P, L], F32,
                                                 name=f"sc{pair}_{mt}_{hh}", tag="sc")
                                sc.append(s)
                                p0 = hh * DH
                                for lh in range(NH):
                                    nc.tensor.matmul(
                                        s[:, lh * 512:(lh + 1) * 512],
                                        khT[p0:p0 + DH, pair, mt * P:(mt + 1) * P],
                                        qhT[p0:p0 + DH, pair, lh * 512:(lh + 1) * 512],
                                        start=True, stop=True)
                            for hh in range(2):
                                pt = ptp.tile([P, L], BF16,
                                              name=f"pt{pair}_{mt}_{hh}", tag="pt")
                                pts[(pair, mt, hh)] = pt
                                nc.scalar.activation(pt, sc[hh], Exp,
                                                     bias=mask_bias[:, mt:mt + 1],
                                                     scale=SCALE)

                    def b3_chunk(mts):
                        for mt in mts:
                            for dh2 in range(NH):
                                ps = psum_b3.tile([P, 512], F32, tag="ps3")
                                for ct in range(KT):
                                    nc.tensor.matmul(
                                        ps, vT[:, mt, ct, :],
                                        WvT[:, dh2 * 4:(dh2 + 1) * 4, ct, :],
                                        start=(ct == 0), stop=(ct == KT - 1))
                                # dst: 4 pair-groups x (even at +0, odd at +128)
                                dst = bass.AP(
                                    tensor=vh_base.tensor,
                                    offset=vh_base.offset + mt * VW + dh2 * 4 * 3 * DH,
                                    ap=[list(vh_base.ap[0]), [3 * DH, 4],
                                        [2 * DH, 2], [1, DH]])
                                bvs = bvb[:, dh2 * 512:(dh2 + 1) * 512].rearrange(
                                    "p (a b d) -> p a b d", b=2, d=DH)
                                nc.vector.tensor_add(
                                    dst,
                                    ps[:].rearrange("p (a b d) -> p a b d",
                                                    b=2, d=DH),
                                    bvs)

                    def attnv(pair):
                        for hh in range(2):
                            h = 2 * pair + hh
                            avs = [psum_av.tile([P, 512], F32,
                                                name=f"av{pair}_{hh}_{lh}",
                                                tag="av")
                                   for lh in range(NH)]
                            for mt in range(LT):
                                for lh in range(NH):
                                    nc.tensor.matmul(
                                        avs[lh],
                                        vh_stat(mt, h),
                                        pts[(pair, mt, hh)][:, lh * 512:(lh + 1) * 512],
                                        start=(mt == 0), stop=(mt == LT - 1))
                            # even head: rows 0:64 = o, 64:128 = den; odd: swapped
                            o_off = 0 if h % 2 == 0 else DH
                            d_off = DH - o_off
                            for lh in range(NH):
                                av = avs[lh]
                                rec = recp.tile([P, 512], F32,
                                                name=f"rec{pair}_{hh}_{lh}",
                                                tag="rec")
                                if d_off != 0:
                                    # custom-DVE recip needs partition base 0
                                    tmp = recp.tile([P, 512], F32,
                                                    name=f"tm{pair}_{hh}_{lh}",
                                                    tag="rec")
                                    nc.vector.tensor_copy(
                                        tmp[0:DH, :], av[d_off:d_off + DH, :])
                                    den = tmp
                                else:
                                    den = av
                                nc.vector.reciprocal_approx_fast(
                                    rec[0:DH, :], den[0:DH, :])
                                nc.vector.tensor_mul(
                                    oT[hh * DH:(hh + 1) * DH, pair,
                                       lh * 512:(lh + 1) * 512],
                                    av[o_off:o_off + DH, :], rec[0:DH, :])
                            for mt in range(LT):
                                del pts[(pair, mt, hh)]

                    scores_exp(0)
                    b3_chunk(range(0, 4))
                    scores_exp(1)
                    b3_chunk(range(4, 8))
                    def wo_t(jts):
                        # Wo fp32 PE transposes mid-attention, sharing the b3
                        # slot (half-rowtile granularity: [P,4,128] f32 = 1 bank)
                        for jt in jts:
                            for half in range(2):
                                ps = psum_b3.tile([P, 4, P], F32, tag="ps3")
                                for cc in range(4):
                                    ct = half * 4 + cc
                                    nc.tensor.matmul(
                                        ps[:, cc, :],
                                        wo_nat[jt][:, 0, ct * P:(ct + 1) * P],
                                        ident32, is_transpose=True)
                                nc.vector.tensor_copy(
                                    WoT[:, jt, half * 4:(half + 1) * 4, :], ps)

                    attnv(0)
                    scores_exp(2)
                    wo_t(range(0, 4))
                    attnv(1)
                    scores_exp(3)
                    wo_t(range(4, 8))
                    attnv(2)
                    for pair in range(4, H // 2):
                        scores_exp(pair)
                        attnv(pair - 1)
                    attnv(H // 2 - 1)

                # ---------------- out-projection + residual + layernorm
                with (
                    tc.tile_pool(name="dwork", bufs=3) as dwork,
                    tc.tile_pool(name="dsmall", bufs=8) as dsmall,
                    tc.tile_pool(name="psum_y", bufs=3, space="PSUM") as psum_y,
                ):
                    for lt in range(LT):
                        yp = psum_y.tile([P, C], F32, tag="yp")
                        ysb = dwork.tile([P, C], F32, tag="ysb")
                        st = dsmall.tile([P, 2, 6], F32, tag="st")
                        for ch in range(NH):
                            for dt in range(DT):
                                nc.tensor.matmul(
                                    yp[:, ch * 512:(ch + 1) * 512],
                                    oT[:, dt, lt * P:(lt + 1) * P],
                                    WoT[:, ch * 4:(ch + 1) * 4, dt, :],
                                    start=(dt == 0), stop=False)
                            nc.tensor.matmul(
                                yp[:, ch * 512:(ch + 1) * 512],
                                ones_row[0:1, :],
                                bo_bf[0:1, ch * 512:(ch + 1) * 512],
                                start=False, stop=True)
                            # evict + stats per 512-half while the other half runs
                            nc.vector.tensor_add(
                                ysb[:, ch * 512:(ch + 1) * 512],
                                yp[:, ch * 512:(ch + 1) * 512],
                                q_nat[:, lt, ch * 512:(ch + 1) * 512])
                            nc.vector.bn_stats(st[:, ch, :],
                                               ysb[:, ch * 512:(ch + 1) * 512])
                        mv = dsmall.tile([P, 2], F32, tag="mv")
                        nc.vector.bn_aggr(mv, st)
                        rstd = dsmall.tile([P, 1], F32, tag="rstd")
                        nc.scalar.activation(rstd, mv[:, 1:2], Sqrt,
                                             bias=eps_sb[:, 0:1])
                        nc.vector.reciprocal(rstd, rstd)
                        nmr = dsmall.tile([P, 1], F32, tag="nmr")
                        nc.vector.tensor_mul(nmr, mv[:, 0:1], rstd)
                        nc.vector.tensor_scalar_mul(nmr, nmr, -1.0)
                        yn = dwork.tile([P, C], F32, tag="yn", bufs=6)
                        nc.vector.tensor_scalar(yn, ysb, rstd[:, 0:1],
                                                nmr[:, 0:1], MULT, ADD)
                        if apply_gb:
                            nc.vector.tensor_mul(yn, yn, gamma_b)
                            nc.gpsimd.tensor_add(yn, yn, beta_b)
                        eng = nc.sync if lt % 2 == 0 else nc.scalar
                        eng.dma_start(y_out[lt * P:(lt + 1) * P, :], yn)

    nc.compile()
    return nc


def _get_nc(apply_gb):
    key = ("nc", apply_gb)
    if key not in _CACHE:
        _CACHE[key] = build(apply_gb)
    return _CACHE[key]


def kernel(**inputs) -> np.ndarray:
    global LAST_RESULT
    gamma = np.asarray(inputs["gamma"], dtype=np.float32)
    beta = np.asarray(inputs["beta"], dtype=np.float32)
    apply_gb = not (np.all(gamma == 1.0) and np.all(beta == 0.0))
    nc = _get_nc(apply_gb)
    q = np.ascontiguousarray(np.asarray(inputs["q"], dtype=np.float32))
    k = np.ascontiguousarray(np.asarray(inputs["k"], dtype=np.float32))
    v = np.ascontiguousarray(np.asarray(inputs["v"], dtype=np.float32))
    mask = np.ascontiguousarray(np.asarray(inputs["key_padding_mask"]).astype(np.uint8))
    shared = {
        name: np.ascontiguousarray(np.asarray(inputs[name], dtype=np.float32))
        for name in ("Wq", "bq", "Wk", "bk", "Wv", "bv", "Wo", "bo", "gamma", "beta")
    }
    in_maps = []
    for b in range(B):
        m = {"q": q[b], "k": k[b], "v": v[b], "key_padding_mask": mask[b]}
        m.update(shared)
        in_maps.append(m)
    LAST_RESULT = run_bass_kernel_spmd(nc, in_maps, core_ids=list(range(B)), trace=TRACE)
    return np.stack([r["y"] for r in LAST_RESULT.results], axis=0)
